# revision 27
# baseline (speedup 1.0000x reference)
"""Trainium2 Bass kernel: Backprojection3DConsistencyLoss (8-core SPMD).

Contract: kernel(**inputs) takes the FULL unsharded inputs of the reference
(pred_frontal/pred_lateral [2,1,128,128] f32, source/target geometry, the
ground-truth volume [128,128,128] f32, A_inv [3,3], t_inv [3]) and returns the
FULL scalar loss, computing the heavy work on 8 NeuronCores.

Algorithm (separable per-slice reconstruction; no collectives):
  For this module's geometry the detector plane is constant along the scan
  axis, so every ray shares the same scan-axis sample sequence
  z_s = src_z + dz*(2.5 s/511) (the ray-length normalization cancels exactly).
  Slice k therefore has a unique shared sample s_k (or none), and within the
  slice the hit voxel is a separable affine map of the detector indices:
  y = rint(sy + (j-sy)*a_k), x = rint(sx + (i-sx)*a_k) with a_k = 2.5 s_k/511.
  With one-hot matrices A_k[p, v] = [map_k(p) == v] the 0/1 slice image is
  sign(A_k^T . mask^T-ish . A_k) — two small matmuls on the TensorEngine.

  Sharding: core c owns z-slices [16c, 16c+16) of all four volumes.
  Frontal volumes are built slice-by-slice (z = slice axis); lateral volumes
  (x = slice axis) are built as the 16-column z-band of every x-slice, via a
  skinny pair of matmuls per non-empty x-slice.  All per-core variation is
  carried by host-computed bf16 lookup tables, so one SPMD program serves all
  8 cores and no ReduceScatter is needed.  Each core evaluates its BCE shard
  with the exact quadratic q0 + q1*s + q2*s^2 + gt*s (s in {0,1,2}) reduced
  on-device to a few per-partition sums; the host combines them.

If the input geometry violates the separability assumptions (it holds for
this module's detector geometry; margins are checked in f64), a faithful f32
numpy fallback computes the same result on host.
"""

import math
import sys

import numpy as np

for _p in ("/opt/trn_rl_repo",):
    if _p not in sys.path:
        sys.path.insert(0, _p)

import ml_dtypes  # noqa: E402

import concourse.bacc as bacc  # noqa: E402
import concourse.mybir as mybir  # noqa: E402
import concourse.tile as tile  # noqa: E402
from concourse.bass_utils import run_bass_kernel_spmd  # noqa: E402

N_CORES = 8
V = 128          # volume side
S = 512          # samples per ray
ZW = V // N_CORES  # z-slices per core (16)
POISON = 200.0   # one-hot compare value that never matches a voxel coordinate
F32 = mybir.dt.float32
BF16 = mybir.dt.bfloat16
I32 = mybir.dt.int32
ALU = mybir.AluOpType

# BCE quadratic: cell loss = q0 + q1*s + q2*s^2 + gt*s, exact for s in {0,1,2}
_B0 = math.log(0.5)
_B1 = -math.log1p(math.e)
_B2 = -2.0 - math.log1p(math.exp(-2.0))
Q0 = _B0
Q1 = (-3.0 * _B0 + 4.0 * _B1 - _B2) / 2.0
Q2 = (_B0 - 2.0 * _B1 + _B2) / 2.0

# packed bf16 table layout (columns of the [128, W] "tabs" input); lateral
# tables carry only the NL non-empty x-slices, in nl_ks order
_C_VXF = 0      # [128, 16]  frontal x map for the core's 16 z-slices
_C_VYF = 16     # [128, 16]  frontal y map
_C_VYL = 32     # [128, NL]  lateral y map
# _C_VZL = 32+NL  [128, NL]  lateral z map, shifted by -16c
# _C_MF  = 32+2NL [128, 2*128] frontal masks (i-partition, j-free)
# _C_MLT = +2*128 [128, 2*128] lateral masks transposed (j-part, i-free)


def _tab_offsets(nl):
    c_vzl = _C_VYL + nl
    c_mf = c_vzl + nl
    c_mlt = c_mf + 2 * V
    return c_vzl, c_mf, c_mlt, c_mlt + 2 * V

_PROGRAM_CACHE: dict = {}


class _GeometryFallback(Exception):
    pass


def _build_program(nl_ks: tuple):
    key = nl_ks
    if key in _PROGRAM_CACHE:
        return _PROGRAM_CACHE[key]

    nl = len(nl_ks)
    c_vzl, c_mf, c_mlt, tabs_w = _tab_offsets(nl)
    nc = bacc.Bacc("TRN2", target_bir_lowering=False, debug=False,
                   num_devices=N_CORES)
    tabs = nc.declare_dram_parameter("tabs", [128, tabs_w], BF16,
                                     isOutput=False)
    gt_p = nc.declare_dram_parameter("gt", [128, ZW * V], BF16, isOutput=False)
    out_p = nc.declare_dram_parameter("out_vec", [128, 8], F32, isOutput=True)

    with tile.TileContext(nc) as tc:
        with (
            tc.tile_pool(name="const", bufs=1) as constp,
            tc.tile_pool(name="vsb", bufs=3) as vsbp,
            tc.tile_pool(name="wsb", bufs=2) as wsbp,
            tc.tile_pool(name="bce", bufs=2) as bcep,
            tc.tile_pool(name="psV", bufs=2, space="PSUM") as psVp,
            tc.tile_pool(name="psCF", bufs=2, space="PSUM") as psCFp,
            tc.tile_pool(name="psW", bufs=2, space="PSUM") as psWp,
            tc.tile_pool(name="psC", bufs=2, space="PSUM") as psCp,
        ):
            tabs_sb = constp.tile([128, tabs_w], BF16)
            nc.sync.dma_start(tabs_sb[:], tabs.ap())
            gt_sb = constp.tile([128, ZW, V], BF16)
            nc.sync.dma_start(gt_sb[:], gt_p.ap())

            iota_i = constp.tile([128, V], I32)
            nc.gpsimd.iota(iota_i[:], pattern=[[1, V]], base=0,
                           channel_multiplier=0)
            iota_b = constp.tile([128, V], BF16)
            nc.vector.tensor_copy(iota_b[:], iota_i[:])

            def one_hot(name, val_ap, nk, nv, eng=None):
                """AT[p, k, v] = (val[p, k] == v), bf16 {0,1}."""
                t = constp.tile([128, nk, nv], BF16, tag=name)
                (eng or nc.vector).tensor_tensor(
                    t[:],
                    iota_b[:, 0:nv].unsqueeze(1).broadcast_to([128, nk, nv]),
                    val_ap.unsqueeze(2).broadcast_to([128, nk, nv]),
                    ALU.is_equal)
                return t

            # lateral one-hots first: they gate the PE-dominant lateral pass.
            # ATyL is built in 16-slice chunks so the first lateral block's
            # mm2 isn't gated on the whole build.
            ATzL = one_hot("ATzL", tabs_sb[:, c_vzl:c_vzl + nl], nl, ZW)
            ATyL = constp.tile([128, nl, V], BF16, tag="ATyL")
            for p0 in range(0, nl, 16):
                pw = min(16, nl - p0)
                nc.vector.tensor_tensor(
                    ATyL[:, p0:p0 + pw, :],
                    iota_b[:].unsqueeze(1).broadcast_to([128, pw, V]),
                    tabs_sb[:, _C_VYL + p0:_C_VYL + p0 + pw]
                    .unsqueeze(2).broadcast_to([128, pw, V]),
                    ALU.is_equal)
            ATxF = one_hot("ATxF", tabs_sb[:, _C_VXF:_C_VXF + ZW], ZW, V)
            ATyF = one_hot("ATyF", tabs_sb[:, _C_VYF:_C_VYF + ZW], ZW, V)

            def maskF(b):   # [i, j]
                return tabs_sb[:, c_mf + V * b:c_mf + V * (b + 1)]

            def maskLT(b):  # [j, i]
                return tabs_sb[:, c_mlt + V * b:c_mlt + V * (b + 1)]

            volF = constp.tile([128, 2, ZW, V], BF16)
            volL = constp.tile([128, 2, ZW, V], BF16)
            nc.gpsimd.memset(volL[:], 0.0)

            # ---- lateral: [y, z-band] columns of each non-empty x-slice,
            #      batched 16 slices per PSUM bank; mm1 is one wide matmul
            #      per (block, batch) since its weights (the mask) are fixed
            LB = 16
            for blk in range(0, nl, LB):
                ks = nl_ks[blk:blk + LB]
                nb = len(ks)
                psW = psWp.tile([128, 2, LB, ZW], F32)
                for b in range(2):
                    nc.tensor.matmul(psW[:, b, 0:nb, :], lhsT=maskLT(b),
                                     rhs=ATzL[:, blk:blk + nb, :],
                                     start=True, stop=True)
                wsb = wsbp.tile([128, 2, LB, ZW], BF16, tag="w")
                nc.scalar.copy(wsb[:, :, 0:nb, :], psW[:, :, 0:nb, :])
                psC2 = psCp.tile([128, LB, 2, ZW], F32)
                for slot, k in enumerate(ks):
                    nc.tensor.matmul(psC2[:, slot], lhsT=ATyL[:, blk + slot, :],
                                     rhs=wsb[:, :, slot, :],
                                     start=True, stop=True)
                for slot, k in enumerate(ks):
                    nc.scalar.sign(volL[:, :, :, k], psC2[:, slot])

            # ---- frontal: full [y, x] slice per owned z, mm1 two slices
            #      wide per batch ----
            FB = 2
            for k0 in range(0, ZW, FB):
                psV = psVp.tile([128, 2, FB, V], F32)
                for b in range(2):
                    nc.tensor.matmul(psV[:, b], lhsT=maskF(b),
                                     rhs=ATxF[:, k0:k0 + FB, :],
                                     start=True, stop=True)
                vsb = vsbp.tile([128, 2, FB, V], BF16, tag="v")
                if (k0 // FB) % 2 == 0:
                    nc.vector.tensor_copy(vsb[:], psV[:])
                else:
                    nc.scalar.copy(vsb[:], psV[:])
                for kl in range(FB):
                    kk = k0 + kl
                    psC = psCFp.tile([128, 2, V], F32)
                    nc.tensor.matmul(psC[:], lhsT=ATyF[:, kk, :],
                                     rhs=vsb[:, :, kl, :],
                                     start=True, stop=True)
                    nc.scalar.sign(volF[:, :, kk, :], psC[:])

            # ---- BCE partial sums; out cols per batch b:
            #      4b+0 = sum(min(s,1)), 4b+1 = sum(s==2), 4b+2 = sum(gt*s)
            #      (sum(s) = col0 + col1).  All ops are tensor_tensor /
            #      tensor_scalar so the DVE 2x bf16 mode applies. ----
            out_sb = constp.tile([128, 8], F32)
            nc.gpsimd.memset(out_sb[:], 0.0)
            for b in range(2):
                s = bcep.tile([128, ZW, V], BF16, tag="s")
                nc.vector.tensor_tensor(s[:], volF[:, b], volL[:, b], ALU.add)
                s1 = bcep.tile([128, ZW, V], BF16, tag="s1")
                nc.vector.tensor_scalar(
                    s1[:], s[:], 1.0, 0.0, ALU.min, ALU.add,
                    accum_out=out_sb[:, 4 * b:4 * b + 1])
                e2 = bcep.tile([128, ZW, V], BF16, tag="e2")
                nc.vector.tensor_scalar(
                    e2[:], s[:], 2.0, 0.0, ALU.is_equal, ALU.add,
                    accum_out=out_sb[:, 4 * b + 1:4 * b + 2])
                gs = bcep.tile([128, ZW, V], BF16, tag="gs")
                nc.vector.tensor_tensor(gs[:], gt_sb[:], s[:], ALU.mult)
                g2 = bcep.tile([128, ZW, V], BF16, tag="g2")
                nc.vector.tensor_scalar(
                    g2[:], gs[:], 0.0, 0.0, ALU.add, ALU.add,
                    accum_out=out_sb[:, 4 * b + 2:4 * b + 3])
            nc.sync.dma_start(out_p.ap(), out_sb[:])

    nc.compile()
    _PROGRAM_CACHE[key] = nc
    return nc


def _host_prep(inputs):
    """Validate geometry and build per-core bf16 tables.

    Returns (in_maps, nl_ks).  Raises _GeometryFallback when the separability
    assumptions don't hold.
    """
    f32 = np.float32
    pf = np.asarray(inputs["pred_frontal"], dtype=f32)
    pl = np.asarray(inputs["pred_lateral"], dtype=f32)
    srcF = np.asarray(inputs["source_F"], dtype=np.float64)[0]
    tgtF = np.asarray(inputs["target_F"], dtype=np.float64)[0]
    srcL = np.asarray(inputs["source_L"], dtype=np.float64)[0]
    tgtL = np.asarray(inputs["target_L"], dtype=np.float64)[0]
    A_inv = np.asarray(inputs["A_inv"], dtype=np.float64)
    t_inv = np.asarray(inputs["t_inv"], dtype=np.float64)
    gt = np.asarray(inputs["vol_gt_3d"], dtype=f32)
    B = pf.shape[0]
    if B != 2 or gt.shape != (V, V, V) or pf.shape[2:] != (V, V):
        raise _GeometryFallback(f"unexpected shapes B={B}")
    if not np.array_equal(A_inv, np.diag(np.diag(A_inv))):
        raise _GeometryFallback("A_inv not diagonal")
    D = np.diag(A_inv)

    def view_tables(src, tgt, scan_ax, ax_i, ax_j):
        """Per-slice sample index + separable coordinate maps (f64)."""
        # target coordinate along scan axis must be globally constant;
        # along ax_i it may depend only on detector row i, ax_j only on j.
        c = tgt[0, 0, scan_ax]
        if not np.all(tgt[..., scan_ax] == c):
            raise _GeometryFallback("scan axis not constant")
        ti = tgt[:, 0, ax_i]
        if not np.all(tgt[..., ax_i] == ti[:, None]):
            raise _GeometryFallback("ax_i not separable")
        tj = tgt[0, :, ax_j]
        if not np.all(tgt[..., ax_j] == tj[None, :]):
            raise _GeometryFallback("ax_j not separable")

        beta = 2.5 * np.arange(S, dtype=np.float64) / (S - 1.0)
        zeta = (src[scan_ax] + (c - src[scan_ax]) * beta) * D[scan_ax] \
            + t_inv[scan_ax]
        ks = np.rint(zeta).astype(np.int64)
        margin = np.abs(np.abs(zeta - np.rint(zeta)) - 0.5).min()
        if margin < 5e-4:
            raise _GeometryFallback(f"scan margin {margin:.1e}")
        inb = (ks >= 0) & (ks < V)
        if len(np.unique(ks[inb])) != int(inb.sum()):
            raise _GeometryFallback("multiple samples per slice")
        s_for_k = np.full(V, -1, np.int64)
        s_for_k[ks[inb]] = np.arange(S)[inb]

        p = np.arange(V, dtype=np.float64)

        def cmap(tvals, axis):
            """[p, k] voxel coordinate map with POISON for invalid entries."""
            out = np.full((V, V), POISON, dtype=np.float64)
            for k in range(V):
                sk = s_for_k[k]
                if sk < 0:
                    continue
                a = beta[sk]
                w = (src[axis] + (tvals - src[axis]) * a) * D[axis] \
                    + t_inv[axis]
                m = np.abs(np.abs(w - np.rint(w)) - 0.5).min()
                if m < 5e-4:
                    raise _GeometryFallback(f"transverse margin {m:.1e}")
                r = np.rint(w)
                r[(r < 0) | (r >= V)] = POISON
                out[:, k] = r
            return out

        return s_for_k, cmap(ti, ax_i), cmap(tj, ax_j)

    # frontal: scan z(2), i -> vol axis 0 (x), j -> vol axis 1 (y)
    sfF, mapxF, mapyF = view_tables(srcF, tgtF, 2, 0, 1)
    # lateral: scan x(0), i -> vol axis 1 (y), j -> vol axis 2 (z)
    sfL, mapyL, mapzL = view_tables(srcL, tgtL, 0, 1, 2)

    nl_ks = tuple(int(k) for k in range(V) if sfL[k] >= 0)
    if not nl_ks:
        nl_ks = (0,)  # degenerate but keeps the program shape valid

    bf16 = ml_dtypes.bfloat16
    nl = len(nl_ks)
    c_vzl, c_mf, c_mlt, tabs_w = _tab_offsets(nl)
    klist = np.array(nl_ks, dtype=np.int64)
    maskF = (pf[:, 0] > 0.5)                       # [b, i, j]
    maskLT = (pl[:, 0] > 0.5).transpose(0, 2, 1)    # [b, j, i]
    gtzyx = np.ascontiguousarray(gt.transpose(1, 2, 0))  # [y][z][x]

    in_maps = []
    for cidx in range(N_CORES):
        z0 = ZW * cidx
        tabs = np.full((128, tabs_w), POISON, dtype=np.float64)
        tabs[:, _C_VXF:_C_VXF + ZW] = mapxF[:, z0:z0 + ZW]
        tabs[:, _C_VYF:_C_VYF + ZW] = mapyF[:, z0:z0 + ZW]
        tabs[:, _C_VYL:_C_VYL + nl] = mapyL[:, klist]
        vz = mapzL[:, klist].copy()
        ok = vz != POISON
        vz[ok] = vz[ok] - z0
        tabs[:, c_vzl:c_vzl + nl] = vz
        tabs[:, c_mf:c_mf + V] = maskF[0]
        tabs[:, c_mf + V:c_mf + 2 * V] = maskF[1]
        tabs[:, c_mlt:c_mlt + V] = maskLT[0]
        tabs[:, c_mlt + V:c_mlt + 2 * V] = maskLT[1]
        gshard = gtzyx[:, z0:z0 + ZW, :].reshape(128, ZW * V)
        in_maps.append({"tabs": tabs.astype(bf16),
                        "gt": gshard.astype(bf16)})
    return in_maps, nl_ks


def _combine(results) -> np.ndarray:
    """Host-side reduction of the 8 per-core [128, 8] partial-sum tensors."""
    acc = np.zeros(8, dtype=np.float64)
    for r in results:
        acc += np.asarray(r["out_vec"], dtype=np.float64).sum(axis=0)
    total = 0.0
    for b in range(2):
        ss = acc[4 * b] + acc[4 * b + 1]   # sum(s) = sum(min(s,1)) + sum(s==2)
        se2, sgs = acc[4 * b + 1], acc[4 * b + 2]
        total += Q0 * (V ** 3) + (Q1 + Q2) * ss + 2.0 * Q2 * se2 + sgs
    return np.float32(-total / (2.0 * V ** 3))


def _reference_fallback(inputs):
    """Faithful f32 numpy replica of the jax reference (safety net)."""
    f32 = np.float32
    pf = np.asarray(inputs["pred_frontal"], dtype=f32)
    pl = np.asarray(inputs["pred_lateral"], dtype=f32)
    srcF = np.asarray(inputs["source_F"], dtype=f32)[0]
    tgtF = np.asarray(inputs["target_F"], dtype=f32)[0]
    srcL = np.asarray(inputs["source_L"], dtype=f32)[0]
    tgtL = np.asarray(inputs["target_L"], dtype=f32)[0]
    A_inv = np.asarray(inputs["A_inv"], dtype=f32)
    t_inv = np.asarray(inputs["t_inv"], dtype=f32)
    gt = np.asarray(inputs["vol_gt_3d"], dtype=f32)

    def backproject(mask2d, src, tgt):
        active = (mask2d > 0.5).reshape(-1)
        det = tgt.reshape(-1, 3).astype(f32)
        rd = (det - src[None, :]).astype(f32)
        rl = np.sqrt((rd * rd).sum(1, dtype=f32)).astype(f32)[:, None]
        rdn = (rd / (rl + f32(1e-8))).astype(f32)
        tv = (np.arange(S, dtype=f32) * (f32(1.0) / f32(S - 1)))
        ts = (tv[None, :, None] * (rl[:, None, :] * f32(2.5))).astype(f32)
        world = (src[None, None, :] + rdn[:, None, :] * ts).astype(f32)
        vox_f = (world @ A_inv.T + t_inv).astype(f32)
        vox = np.rint(vox_f).astype(np.int64)
        ok = (active[:, None]
              & (vox[..., 0] >= 0) & (vox[..., 0] < V)
              & (vox[..., 1] >= 0) & (vox[..., 1] < V)
              & (vox[..., 2] >= 0) & (vox[..., 2] < V))
        vi = np.clip(vox, 0, V - 1)
        vol = np.zeros((V, V, V), dtype=f32)
        flat = (vi[..., 0] * V + vi[..., 1]) * V + vi[..., 2]
        vol.reshape(-1)[flat[ok]] = 1.0
        return vol

    total = 0.0
    B = pf.shape[0]
    for b in range(B):
        vF = backproject(pf[b, 0], srcF, tgtF)
        vL = backproject(pl[b, 0], srcL, tgtL)
        sv = (vF + vL).astype(np.float64)
        p = 1.0 / (1.0 + np.exp(-sv))
        total += -(gt * np.log(p) + (1.0 - gt) * np.log1p(-p)).mean()
    return np.float32(total / B)


def kernel(**inputs) -> np.ndarray:
    try:
        in_maps, nl_ks = _host_prep(inputs)
    except _GeometryFallback:
        return _reference_fallback(inputs)
    nc = _build_program(nl_ks)
    res = run_bass_kernel_spmd(nc, in_maps, list(range(N_CORES)))
    return _combine(res.results)


# revision 32
# speedup vs baseline: 1.1932x; 1.1932x over previous
"""Trainium2 Bass kernel: Backprojection3DConsistencyLoss (8-core SPMD).

Contract: kernel(**inputs) takes the FULL unsharded inputs of the reference
(pred_frontal/pred_lateral [2,1,128,128] f32, source/target geometry, the
ground-truth volume [128,128,128] f32, A_inv [3,3], t_inv [3]) and returns the
FULL scalar loss, computing the heavy work on 8 NeuronCores.

Algorithm (separable per-slice reconstruction; no collectives):
  For this module's geometry the detector plane is constant along the scan
  axis, so every ray shares the same scan-axis sample sequence
  z_s = src_z + dz*(2.5 s/511) (the ray-length normalization cancels exactly).
  Slice k therefore has a unique shared sample s_k (or none), and within the
  slice the hit voxel is a separable affine map of the detector indices:
  y = rint(sy + (j-sy)*a_k), x = rint(sx + (i-sx)*a_k) with a_k = 2.5 s_k/511.
  With one-hot matrices A_k[p, v] = [map_k(p) == v] the 0/1 slice image is
  sign(A_k^T . mask^T-ish . A_k) — two small matmuls on the TensorEngine.

  Sharding: core c owns z-slices [16c, 16c+16) of all four volumes.
  Frontal volumes are built slice-by-slice (z = slice axis); lateral volumes
  (x = slice axis) are built as the 16-column z-band of every x-slice, via a
  skinny pair of matmuls per non-empty x-slice.  All per-core variation is
  carried by host-computed bf16 lookup tables, so one SPMD program serves all
  8 cores and no ReduceScatter is needed.  Each core evaluates its BCE shard
  with the exact quadratic q0 + q1*s + q2*s^2 + gt*s (s in {0,1,2}) reduced
  on-device to a few per-partition sums; the host combines them.

If the input geometry violates the separability assumptions (it holds for
this module's detector geometry; margins are checked in f64), a faithful f32
numpy fallback computes the same result on host.
"""

import math
import sys

import numpy as np

for _p in ("/opt/trn_rl_repo",):
    if _p not in sys.path:
        sys.path.insert(0, _p)

import ml_dtypes  # noqa: E402

import concourse.bacc as bacc  # noqa: E402
import concourse.mybir as mybir  # noqa: E402
import concourse.tile as tile  # noqa: E402
from concourse.bass_utils import run_bass_kernel_spmd  # noqa: E402

N_CORES = 8
V = 128          # volume side
S = 512          # samples per ray
ZW = V // N_CORES  # z-slices per core (16)
POISON = 200.0   # one-hot compare value that never matches a voxel coordinate
F32 = mybir.dt.float32
BF16 = mybir.dt.bfloat16
I32 = mybir.dt.int32
ALU = mybir.AluOpType

# BCE quadratic: cell loss = q0 + q1*s + q2*s^2 + gt*s, exact for s in {0,1,2}
_B0 = math.log(0.5)
_B1 = -math.log1p(math.e)
_B2 = -2.0 - math.log1p(math.exp(-2.0))
Q0 = _B0
Q1 = (-3.0 * _B0 + 4.0 * _B1 - _B2) / 2.0
Q2 = (_B0 - 2.0 * _B1 + _B2) / 2.0

# packed bf16 table layout (columns of the [128, W] "tabs" input); lateral
# tables carry only the NL non-empty x-slices, in nl_ks order
_C_VXF = 0      # [128, 16]  frontal x map for the core's 16 z-slices
_C_VYF = 16     # [128, 16]  frontal y map
_C_VYL = 32     # [128, NL]  lateral y map
# _C_VZL = 32+NL  [128, NL]  lateral z map, shifted by -16c
# _C_MF  = 32+2NL [128, 2*128] frontal masks (i-partition, j-free)
# _C_MLT = +2*128 [128, 2*128] lateral masks transposed (j-part, i-free)


def _tab_offsets(nl):
    c_vzl = _C_VYL + nl
    c_mf = c_vzl + nl
    c_mlt = c_mf + 2 * V
    c_xv = c_mlt + 2 * V
    return c_vzl, c_mf, c_mlt, c_xv, c_xv + V

_PROGRAM_CACHE: dict = {}


class _GeometryFallback(Exception):
    pass


def _build_program(nl_ks: tuple):
    key = nl_ks
    if key in _PROGRAM_CACHE:
        return _PROGRAM_CACHE[key]

    nl = len(nl_ks)
    c_vzl, c_mf, c_mlt, c_xv, tabs_w = _tab_offsets(nl)
    nc = bacc.Bacc("TRN2", target_bir_lowering=False, debug=False,
                   num_devices=N_CORES)
    tabs = nc.declare_dram_parameter("tabs", [128, tabs_w], BF16,
                                     isOutput=False)
    gt_p = nc.declare_dram_parameter("gt", [128, ZW * V], BF16, isOutput=False)
    out_p = nc.declare_dram_parameter("out_vec", [128, 8], F32, isOutput=True)

    with tile.TileContext(nc) as tc:
        with (
            tc.tile_pool(name="const", bufs=1) as constp,
            tc.tile_pool(name="vsb", bufs=3) as vsbp,
            tc.tile_pool(name="wsb", bufs=2) as wsbp,
            tc.tile_pool(name="bce", bufs=2) as bcep,
            tc.tile_pool(name="psV", bufs=2, space="PSUM") as psVp,
            tc.tile_pool(name="psCF", bufs=2, space="PSUM") as psCFp,
            tc.tile_pool(name="psW", bufs=2, space="PSUM") as psWp,
            tc.tile_pool(name="psC", bufs=2, space="PSUM") as psCp,
        ):
            tabs_sb = constp.tile([128, tabs_w], BF16)
            nc.sync.dma_start(tabs_sb[:], tabs.ap())
            gt_sb = constp.tile([128, ZW, V], BF16)
            nc.sync.dma_start(gt_sb[:], gt_p.ap())

            iota_i = constp.tile([128, V], I32)
            nc.gpsimd.iota(iota_i[:], pattern=[[1, V]], base=0,
                           channel_multiplier=0)
            iota_b = constp.tile([128, V], BF16)
            nc.vector.tensor_copy(iota_b[:], iota_i[:])

            def one_hot(name, val_ap, nk, nv, eng=None):
                """AT[p, k, v] = (val[p, k] == v), bf16 {0,1}."""
                t = constp.tile([128, nk, nv], BF16, tag=name)
                (eng or nc.vector).tensor_tensor(
                    t[:],
                    iota_b[:, 0:nv].unsqueeze(1).broadcast_to([128, nk, nv]),
                    val_ap.unsqueeze(2).broadcast_to([128, nk, nv]),
                    ALU.is_equal)
                return t

            # lateral one-hots first: they gate the PE-dominant lateral pass.
            # ATyL is built in 16-slice chunks so the first lateral block's
            # mm2 isn't gated on the whole build.
            ATzL = one_hot("ATzL", tabs_sb[:, c_vzl:c_vzl + nl], nl, ZW)
            ATyL = constp.tile([128, nl, V], BF16, tag="ATyL")
            for p0 in range(0, nl, 16):
                pw = min(16, nl - p0)
                nc.vector.tensor_tensor(
                    ATyL[:, p0:p0 + pw, :],
                    iota_b[:].unsqueeze(1).broadcast_to([128, pw, V]),
                    tabs_sb[:, _C_VYL + p0:_C_VYL + p0 + pw]
                    .unsqueeze(2).broadcast_to([128, pw, V]),
                    ALU.is_equal)
            # ATxF compares against the permuted x-identity row (the volume
            # x-axis is stored with the nl_ks slices first, so lateral signs
            # can batch over contiguous columns)
            ATxF = constp.tile([128, ZW, V], BF16, tag="ATxF")
            nc.vector.tensor_tensor(
                ATxF[:],
                tabs_sb[:, c_xv:c_xv + V].unsqueeze(1)
                .broadcast_to([128, ZW, V]),
                tabs_sb[:, _C_VXF:_C_VXF + ZW].unsqueeze(2)
                .broadcast_to([128, ZW, V]),
                ALU.is_equal)
            ATyF = one_hot("ATyF", tabs_sb[:, _C_VYF:_C_VYF + ZW], ZW, V)

            def maskF(b):   # [i, j]
                return tabs_sb[:, c_mf + V * b:c_mf + V * (b + 1)]

            def maskLT(b):  # [j, i]
                return tabs_sb[:, c_mlt + V * b:c_mlt + V * (b + 1)]

            volF = constp.tile([128, 2, ZW, V], BF16)
            volL = constp.tile([128, 2, ZW, V], BF16)
            nc.gpsimd.memset(volL[:], 0.0)

            # ---- lateral: [y, z-band] columns of each non-empty x-slice,
            #      batched 16 slices per PSUM bank; mm1 is one wide matmul
            #      per (block, batch) since its weights (the mask) are fixed
            LB = 16
            for blk in range(0, nl, LB):
                ks = nl_ks[blk:blk + LB]
                nb = len(ks)
                psW = psWp.tile([128, 2, LB, ZW], F32)
                for b in range(2):
                    nc.tensor.matmul(psW[:, b, 0:nb, :], lhsT=maskLT(b),
                                     rhs=ATzL[:, blk:blk + nb, :],
                                     start=True, stop=True)
                wsb = wsbp.tile([128, 2, LB, ZW], BF16, tag="w")
                nc.scalar.copy(wsb[:, :, 0:nb, :], psW[:, :, 0:nb, :])
                psC2 = psCp.tile([128, LB, 2, ZW], F32)
                for slot, k in enumerate(ks):
                    nc.tensor.matmul(psC2[:, slot], lhsT=ATyL[:, blk + slot, :],
                                     rhs=wsb[:, :, slot, :],
                                     start=True, stop=True)
                # x-permuted layout: slices of this block sit in contiguous
                # volume columns [blk, blk+nb) -> one batched sign
                nc.scalar.sign(
                    volL[:, :, :, blk:blk + nb].transpose([0, 3, 1, 2]),
                    psC2[:, 0:nb])

            # ---- frontal: full [y, x] slice per owned z, mm1 two slices
            #      wide per batch ----
            FB = 2
            for k0 in range(0, ZW, FB):
                psV = psVp.tile([128, 2, FB, V], F32)
                for b in range(2):
                    nc.tensor.matmul(psV[:, b], lhsT=maskF(b),
                                     rhs=ATxF[:, k0:k0 + FB, :],
                                     start=True, stop=True)
                vsb = vsbp.tile([128, 2, FB, V], BF16, tag="v")
                if (k0 // FB) % 2 == 0:
                    nc.vector.tensor_copy(vsb[:], psV[:])
                else:
                    nc.scalar.copy(vsb[:], psV[:])
                for kl in range(FB):
                    kk = k0 + kl
                    psC = psCFp.tile([128, 2, V], F32)
                    nc.tensor.matmul(psC[:], lhsT=ATyF[:, kk, :],
                                     rhs=vsb[:, :, kl, :],
                                     start=True, stop=True)
                    nc.scalar.sign(volF[:, :, kk, :], psC[:])

            # ---- BCE partial sums; out cols per batch b:
            #      4b+0 = sum(min(s,1)), 4b+1 = sum(s==2), 4b+2 = sum(gt*s)
            #      (sum(s) = col0 + col1).  All ops are tensor_tensor /
            #      tensor_scalar so the DVE 2x bf16 mode applies. ----
            out_sb = constp.tile([128, 8], F32)
            nc.gpsimd.memset(out_sb[:], 0.0)
            for b in range(2):
                s = bcep.tile([128, ZW, V], BF16, tag="s")
                nc.vector.tensor_tensor(s[:], volF[:, b], volL[:, b], ALU.add)
                s1 = bcep.tile([128, ZW, V], BF16, tag="s1")
                nc.vector.tensor_scalar(
                    s1[:], s[:], 1.0, 0.0, ALU.min, ALU.add,
                    accum_out=out_sb[:, 4 * b:4 * b + 1])
                e2 = bcep.tile([128, ZW, V], BF16, tag="e2")
                nc.vector.tensor_scalar(
                    e2[:], s[:], 2.0, 0.0, ALU.is_equal, ALU.add,
                    accum_out=out_sb[:, 4 * b + 1:4 * b + 2])
                gs = bcep.tile([128, ZW, V], BF16, tag="gs")
                nc.vector.tensor_tensor(gs[:], gt_sb[:], s[:], ALU.mult)
                g2 = bcep.tile([128, ZW, V], BF16, tag="g2")
                nc.vector.tensor_scalar(
                    g2[:], gs[:], 0.0, 0.0, ALU.add, ALU.add,
                    accum_out=out_sb[:, 4 * b + 2:4 * b + 3])
            nc.sync.dma_start(out_p.ap(), out_sb[:])

    nc.compile()
    _PROGRAM_CACHE[key] = nc
    return nc


def _host_prep(inputs):
    """Validate geometry and build per-core bf16 tables.

    Returns (in_maps, nl_ks).  Raises _GeometryFallback when the separability
    assumptions don't hold.
    """
    f32 = np.float32
    pf = np.asarray(inputs["pred_frontal"], dtype=f32)
    pl = np.asarray(inputs["pred_lateral"], dtype=f32)
    srcF = np.asarray(inputs["source_F"], dtype=np.float64)[0]
    tgtF = np.asarray(inputs["target_F"], dtype=np.float64)[0]
    srcL = np.asarray(inputs["source_L"], dtype=np.float64)[0]
    tgtL = np.asarray(inputs["target_L"], dtype=np.float64)[0]
    A_inv = np.asarray(inputs["A_inv"], dtype=np.float64)
    t_inv = np.asarray(inputs["t_inv"], dtype=np.float64)
    gt = np.asarray(inputs["vol_gt_3d"], dtype=f32)
    B = pf.shape[0]
    if B != 2 or gt.shape != (V, V, V) or pf.shape[2:] != (V, V):
        raise _GeometryFallback(f"unexpected shapes B={B}")
    if not np.array_equal(A_inv, np.diag(np.diag(A_inv))):
        raise _GeometryFallback("A_inv not diagonal")
    D = np.diag(A_inv)

    def view_tables(src, tgt, scan_ax, ax_i, ax_j):
        """Per-slice sample index + separable coordinate maps (f64)."""
        # target coordinate along scan axis must be globally constant;
        # along ax_i it may depend only on detector row i, ax_j only on j.
        c = tgt[0, 0, scan_ax]
        if not np.all(tgt[..., scan_ax] == c):
            raise _GeometryFallback("scan axis not constant")
        ti = tgt[:, 0, ax_i]
        if not np.all(tgt[..., ax_i] == ti[:, None]):
            raise _GeometryFallback("ax_i not separable")
        tj = tgt[0, :, ax_j]
        if not np.all(tgt[..., ax_j] == tj[None, :]):
            raise _GeometryFallback("ax_j not separable")

        beta = 2.5 * np.arange(S, dtype=np.float64) / (S - 1.0)
        zeta = (src[scan_ax] + (c - src[scan_ax]) * beta) * D[scan_ax] \
            + t_inv[scan_ax]
        ks = np.rint(zeta).astype(np.int64)
        margin = np.abs(np.abs(zeta - np.rint(zeta)) - 0.5).min()
        if margin < 5e-4:
            raise _GeometryFallback(f"scan margin {margin:.1e}")
        inb = (ks >= 0) & (ks < V)
        if len(np.unique(ks[inb])) != int(inb.sum()):
            raise _GeometryFallback("multiple samples per slice")
        s_for_k = np.full(V, -1, np.int64)
        s_for_k[ks[inb]] = np.arange(S)[inb]

        p = np.arange(V, dtype=np.float64)

        def cmap(tvals, axis):
            """[p, k] voxel coordinate map with POISON for invalid entries."""
            out = np.full((V, V), POISON, dtype=np.float64)
            for k in range(V):
                sk = s_for_k[k]
                if sk < 0:
                    continue
                a = beta[sk]
                w = (src[axis] + (tvals - src[axis]) * a) * D[axis] \
                    + t_inv[axis]
                m = np.abs(np.abs(w - np.rint(w)) - 0.5).min()
                if m < 5e-4:
                    raise _GeometryFallback(f"transverse margin {m:.1e}")
                r = np.rint(w)
                r[(r < 0) | (r >= V)] = POISON
                out[:, k] = r
            return out

        return s_for_k, cmap(ti, ax_i), cmap(tj, ax_j)

    # frontal: scan z(2), i -> vol axis 0 (x), j -> vol axis 1 (y)
    sfF, mapxF, mapyF = view_tables(srcF, tgtF, 2, 0, 1)
    # lateral: scan x(0), i -> vol axis 1 (y), j -> vol axis 2 (z)
    sfL, mapyL, mapzL = view_tables(srcL, tgtL, 0, 1, 2)

    nl_ks = tuple(int(k) for k in range(V) if sfL[k] >= 0)
    if not nl_ks:
        nl_ks = (0,)  # degenerate but keeps the program shape valid

    bf16 = ml_dtypes.bfloat16
    nl = len(nl_ks)
    c_vzl, c_mf, c_mlt, c_xv, tabs_w = _tab_offsets(nl)
    klist = np.array(nl_ks, dtype=np.int64)
    # x-axis permutation: the nl_ks slices first, the rest after
    xorder = np.concatenate(
        [klist, np.setdiff1d(np.arange(V, dtype=np.int64), klist)])
    maskF = (pf[:, 0] > 0.5)                       # [b, i, j]
    maskLT = (pl[:, 0] > 0.5).transpose(0, 2, 1)    # [b, j, i]
    gtzyx = np.ascontiguousarray(gt.transpose(1, 2, 0))  # [y][z][x]

    in_maps = []
    for cidx in range(N_CORES):
        z0 = ZW * cidx
        tabs = np.full((128, tabs_w), POISON, dtype=np.float64)
        tabs[:, _C_VXF:_C_VXF + ZW] = mapxF[:, z0:z0 + ZW]
        tabs[:, _C_VYF:_C_VYF + ZW] = mapyF[:, z0:z0 + ZW]
        tabs[:, _C_VYL:_C_VYL + nl] = mapyL[:, klist]
        vz = mapzL[:, klist].copy()
        ok = vz != POISON
        vz[ok] = vz[ok] - z0
        tabs[:, c_vzl:c_vzl + nl] = vz
        tabs[:, c_mf:c_mf + V] = maskF[0]
        tabs[:, c_mf + V:c_mf + 2 * V] = maskF[1]
        tabs[:, c_mlt:c_mlt + V] = maskLT[0]
        tabs[:, c_mlt + V:c_mlt + 2 * V] = maskLT[1]
        tabs[:, c_xv:c_xv + V] = xorder[None, :]
        gshard = gtzyx[:, z0:z0 + ZW, :][:, :, xorder].reshape(128, ZW * V)
        in_maps.append({"tabs": tabs.astype(bf16),
                        "gt": np.ascontiguousarray(gshard).astype(bf16)})
    return in_maps, nl_ks


def _combine(results) -> np.ndarray:
    """Host-side reduction of the 8 per-core [128, 8] partial-sum tensors."""
    acc = np.zeros(8, dtype=np.float64)
    for r in results:
        acc += np.asarray(r["out_vec"], dtype=np.float64).sum(axis=0)
    total = 0.0
    for b in range(2):
        ss = acc[4 * b] + acc[4 * b + 1]   # sum(s) = sum(min(s,1)) + sum(s==2)
        se2, sgs = acc[4 * b + 1], acc[4 * b + 2]
        total += Q0 * (V ** 3) + (Q1 + Q2) * ss + 2.0 * Q2 * se2 + sgs
    return np.float32(-total / (2.0 * V ** 3))


def _reference_fallback(inputs):
    """Faithful f32 numpy replica of the jax reference (safety net)."""
    f32 = np.float32
    pf = np.asarray(inputs["pred_frontal"], dtype=f32)
    pl = np.asarray(inputs["pred_lateral"], dtype=f32)
    srcF = np.asarray(inputs["source_F"], dtype=f32)[0]
    tgtF = np.asarray(inputs["target_F"], dtype=f32)[0]
    srcL = np.asarray(inputs["source_L"], dtype=f32)[0]
    tgtL = np.asarray(inputs["target_L"], dtype=f32)[0]
    A_inv = np.asarray(inputs["A_inv"], dtype=f32)
    t_inv = np.asarray(inputs["t_inv"], dtype=f32)
    gt = np.asarray(inputs["vol_gt_3d"], dtype=f32)

    def backproject(mask2d, src, tgt):
        active = (mask2d > 0.5).reshape(-1)
        det = tgt.reshape(-1, 3).astype(f32)
        rd = (det - src[None, :]).astype(f32)
        rl = np.sqrt((rd * rd).sum(1, dtype=f32)).astype(f32)[:, None]
        rdn = (rd / (rl + f32(1e-8))).astype(f32)
        tv = (np.arange(S, dtype=f32) * (f32(1.0) / f32(S - 1)))
        ts = (tv[None, :, None] * (rl[:, None, :] * f32(2.5))).astype(f32)
        world = (src[None, None, :] + rdn[:, None, :] * ts).astype(f32)
        vox_f = (world @ A_inv.T + t_inv).astype(f32)
        vox = np.rint(vox_f).astype(np.int64)
        ok = (active[:, None]
              & (vox[..., 0] >= 0) & (vox[..., 0] < V)
              & (vox[..., 1] >= 0) & (vox[..., 1] < V)
              & (vox[..., 2] >= 0) & (vox[..., 2] < V))
        vi = np.clip(vox, 0, V - 1)
        vol = np.zeros((V, V, V), dtype=f32)
        flat = (vi[..., 0] * V + vi[..., 1]) * V + vi[..., 2]
        vol.reshape(-1)[flat[ok]] = 1.0
        return vol

    total = 0.0
    B = pf.shape[0]
    for b in range(B):
        vF = backproject(pf[b, 0], srcF, tgtF)
        vL = backproject(pl[b, 0], srcL, tgtL)
        sv = (vF + vL).astype(np.float64)
        p = 1.0 / (1.0 + np.exp(-sv))
        total += -(gt * np.log(p) + (1.0 - gt) * np.log1p(-p)).mean()
    return np.float32(total / B)


def kernel(**inputs) -> np.ndarray:
    try:
        in_maps, nl_ks = _host_prep(inputs)
    except _GeometryFallback:
        return _reference_fallback(inputs)
    nc = _build_program(nl_ks)
    res = run_bass_kernel_spmd(nc, in_maps, list(range(N_CORES)))
    return _combine(res.results)


# revision 35
# speedup vs baseline: 1.4373x; 1.2045x over previous
"""Trainium2 Bass kernel: Backprojection3DConsistencyLoss (8-core SPMD).

Contract: kernel(**inputs) takes the FULL unsharded inputs of the reference
(pred_frontal/pred_lateral [2,1,128,128] f32, source/target geometry, the
ground-truth volume [128,128,128] f32, A_inv [3,3], t_inv [3]) and returns the
FULL scalar loss, computing the heavy work on 8 NeuronCores.

Algorithm (separable per-slice reconstruction; no collectives):
  For this module's geometry the detector plane is constant along the scan
  axis, so every ray shares the same scan-axis sample sequence
  z_s = src_z + dz*(2.5 s/511) (the ray-length normalization cancels exactly).
  Slice k therefore has a unique shared sample s_k (or none), and within the
  slice the hit voxel is a separable affine map of the detector indices:
  y = rint(sy + (j-sy)*a_k), x = rint(sx + (i-sx)*a_k) with a_k = 2.5 s_k/511.
  With one-hot matrices A_k[p, v] = [map_k(p) == v] the 0/1 slice image is
  sign(A_k^T . mask^T-ish . A_k) — two small matmuls on the TensorEngine.

  Sharding: core c owns z-slices [16c, 16c+16) of all four volumes.
  Frontal volumes are built slice-by-slice (z = slice axis); lateral volumes
  (x = slice axis) are built as the 16-column z-band of every x-slice, via a
  skinny pair of matmuls per non-empty x-slice.  All per-core variation is
  carried by host-computed bf16 lookup tables, so one SPMD program serves all
  8 cores and no ReduceScatter is needed.  Each core evaluates its BCE shard
  with the exact quadratic q0 + q1*s + q2*s^2 + gt*s (s in {0,1,2}) reduced
  on-device to a few per-partition sums; the host combines them.

If the input geometry violates the separability assumptions (it holds for
this module's detector geometry; margins are checked in f64), a faithful f32
numpy fallback computes the same result on host.
"""

import math
import sys

import numpy as np

for _p in ("/opt/trn_rl_repo",):
    if _p not in sys.path:
        sys.path.insert(0, _p)

import ml_dtypes  # noqa: E402

import concourse.bacc as bacc  # noqa: E402
import concourse.mybir as mybir  # noqa: E402
import concourse.tile as tile  # noqa: E402
from concourse.bass_utils import run_bass_kernel_spmd  # noqa: E402

N_CORES = 8
V = 128          # volume side
S = 512          # samples per ray
ZW = V // N_CORES  # z-slices per core (16)
POISON = 200.0   # one-hot compare value that never matches a voxel coordinate
F32 = mybir.dt.float32
BF16 = mybir.dt.bfloat16
I32 = mybir.dt.int32
ALU = mybir.AluOpType

# BCE quadratic: cell loss = q0 + q1*s + q2*s^2 + gt*s, exact for s in {0,1,2}
_B0 = math.log(0.5)
_B1 = -math.log1p(math.e)
_B2 = -2.0 - math.log1p(math.exp(-2.0))
Q0 = _B0
Q1 = (-3.0 * _B0 + 4.0 * _B1 - _B2) / 2.0
Q2 = (_B0 - 2.0 * _B1 + _B2) / 2.0

# packed bf16 table layout (columns of the [128, W] "tabs" input); lateral
# tables carry only the NL non-empty x-slices, in nl_ks order
_C_VXF = 0      # [128, 16]  frontal x map for the core's 16 z-slices
_C_VYF = 16     # [128, 16]  frontal y map
_C_VYL = 32     # [128, NL]  lateral y map
# _C_VZL = 32+NL  [128, NL]  lateral z map, shifted by -16c
# _C_MF  = 32+2NL [128, 2*128] frontal masks (i-partition, j-free)
# _C_MLT = +2*128 [128, 2*128] lateral masks transposed (j-part, i-free)


def _tab_offsets(nl):
    c_vzl = _C_VYL + nl
    c_mf = c_vzl + nl
    c_mlt = c_mf + 2 * V
    c_xv = c_mlt + 2 * V
    return c_vzl, c_mf, c_mlt, c_xv, c_xv + V

_PROGRAM_CACHE: dict = {}


class _GeometryFallback(Exception):
    pass


def _build_program(nl_ks: tuple):
    key = nl_ks
    if key in _PROGRAM_CACHE:
        return _PROGRAM_CACHE[key]

    nl = len(nl_ks)
    c_vzl, c_mf, c_mlt, c_xv, tabs_w = _tab_offsets(nl)
    nc = bacc.Bacc("TRN2", target_bir_lowering=False, debug=False,
                   num_devices=N_CORES)
    tabs = nc.declare_dram_parameter("tabs", [128, tabs_w], BF16,
                                     isOutput=False)
    gt_p = nc.declare_dram_parameter("gt", [128, ZW * V], BF16, isOutput=False)
    out_p = nc.declare_dram_parameter("out_vec", [128, 32], F32, isOutput=True)

    with tile.TileContext(nc) as tc:
        with (
            tc.tile_pool(name="const", bufs=1) as constp,
            tc.tile_pool(name="vsb", bufs=3) as vsbp,
            tc.tile_pool(name="wsb", bufs=2) as wsbp,
            tc.tile_pool(name="bce", bufs=2) as bcep,
            tc.tile_pool(name="psV", bufs=2, space="PSUM") as psVp,
            tc.tile_pool(name="psCF", bufs=2, space="PSUM") as psCFp,
            tc.tile_pool(name="psW", bufs=2, space="PSUM") as psWp,
            tc.tile_pool(name="psC", bufs=2, space="PSUM") as psCp,
        ):
            tabs_sb = constp.tile([128, tabs_w], BF16)
            nc.sync.dma_start(tabs_sb[:], tabs.ap())
            gt_sb = constp.tile([128, ZW, V], BF16)
            nc.sync.dma_start(gt_sb[:], gt_p.ap())

            iota_i = constp.tile([128, V], I32)
            nc.gpsimd.iota(iota_i[:], pattern=[[1, V]], base=0,
                           channel_multiplier=0)
            iota_b = constp.tile([128, V], BF16)
            nc.vector.tensor_copy(iota_b[:], iota_i[:])

            def one_hot(name, val_ap, nk, nv, eng=None):
                """AT[p, k, v] = (val[p, k] == v), bf16 {0,1}."""
                t = constp.tile([128, nk, nv], BF16, tag=name)
                (eng or nc.vector).tensor_tensor(
                    t[:],
                    iota_b[:, 0:nv].unsqueeze(1).broadcast_to([128, nk, nv]),
                    val_ap.unsqueeze(2).broadcast_to([128, nk, nv]),
                    ALU.is_equal)
                return t

            # lateral one-hots first: they gate the PE-dominant lateral pass.
            # ATyL is built in 16-slice chunks so the first lateral block's
            # mm2 isn't gated on the whole build.
            ATzL = one_hot("ATzL", tabs_sb[:, c_vzl:c_vzl + nl], nl, ZW)
            ATyL = constp.tile([128, nl, V], BF16, tag="ATyL")
            for p0 in range(0, nl, 16):
                pw = min(16, nl - p0)
                nc.vector.tensor_tensor(
                    ATyL[:, p0:p0 + pw, :],
                    iota_b[:].unsqueeze(1).broadcast_to([128, pw, V]),
                    tabs_sb[:, _C_VYL + p0:_C_VYL + p0 + pw]
                    .unsqueeze(2).broadcast_to([128, pw, V]),
                    ALU.is_equal)
            # ATxF compares against the permuted x-identity row (the volume
            # x-axis is stored with the nl_ks slices first, so lateral signs
            # can batch over contiguous columns)
            ATxF = constp.tile([128, ZW, V], BF16, tag="ATxF")
            nc.vector.tensor_tensor(
                ATxF[:],
                tabs_sb[:, c_xv:c_xv + V].unsqueeze(1)
                .broadcast_to([128, ZW, V]),
                tabs_sb[:, _C_VXF:_C_VXF + ZW].unsqueeze(2)
                .broadcast_to([128, ZW, V]),
                ALU.is_equal)
            ATyF = one_hot("ATyF", tabs_sb[:, _C_VYF:_C_VYF + ZW], ZW, V)

            def maskF(b):   # [i, j]
                return tabs_sb[:, c_mf + V * b:c_mf + V * (b + 1)]

            def maskLT(b):  # [j, i]
                return tabs_sb[:, c_mlt + V * b:c_mlt + V * (b + 1)]

            volF = constp.tile([128, 2, ZW, V], BF16)
            volL = constp.tile([128, 2, ZW, V], BF16)
            nc.gpsimd.memset(volL[:], 0.0)

            # ---- lateral: [y, z-band] columns of each non-empty x-slice,
            #      batched 16 slices per PSUM bank; mm1 is one wide matmul
            #      per (block, batch) since its weights (the mask) are fixed
            LB = 16
            for blk in range(0, nl, LB):
                ks = nl_ks[blk:blk + LB]
                nb = len(ks)
                psW = psWp.tile([128, 2, LB, ZW], F32)
                for b in range(2):
                    nc.tensor.matmul(psW[:, b, 0:nb, :], lhsT=maskLT(b),
                                     rhs=ATzL[:, blk:blk + nb, :],
                                     start=True, stop=True)
                wsb = wsbp.tile([128, 2, LB, ZW], BF16, tag="w")
                nc.scalar.copy(wsb[:, :, 0:nb, :], psW[:, :, 0:nb, :])
                psC2 = psCp.tile([128, LB, 2, ZW], F32)
                for slot, k in enumerate(ks):
                    nc.tensor.matmul(psC2[:, slot], lhsT=ATyL[:, blk + slot, :],
                                     rhs=wsb[:, :, slot, :],
                                     start=True, stop=True)
                # x-permuted layout: slices of this block sit in contiguous
                # volume columns [blk, blk+nb) -> one batched sign
                nc.scalar.sign(
                    volL[:, :, :, blk:blk + nb].transpose([0, 3, 1, 2]),
                    psC2[:, 0:nb])

            # ---- frontal: full [y, x] slice per owned z, mm1 two slices
            #      wide per batch.  The BCE is evaluated in four z-quarters
            #      interleaved with the frontal loop (the lateral volume is
            #      already complete), so the reduction runs in DVE idle time
            #      instead of as a serial tail.  out cols per (quarter c,
            #      batch b): 8c+4b+0 = sum(min(s,1)), +1 = sum(s==2),
            #      +2 = sum(gt*s); sum(s) = col0 + col1. ----
            out_sb = constp.tile([128, 32], F32)
            nc.gpsimd.memset(out_sb[:], 0.0)
            QW = ZW // 4   # z-slices per BCE quarter

            def bce_quarter(c):
                z0 = QW * c
                for b in range(2):
                    col = 8 * c + 4 * b
                    s = bcep.tile([128, QW, V], BF16, tag="s")
                    nc.vector.tensor_tensor(s[:], volF[:, b, z0:z0 + QW],
                                            volL[:, b, z0:z0 + QW], ALU.add)
                    s1 = bcep.tile([128, QW, V], BF16, tag="s1")
                    nc.vector.tensor_scalar(
                        s1[:], s[:], 1.0, 0.0, ALU.min, ALU.add,
                        accum_out=out_sb[:, col:col + 1])
                    e2 = bcep.tile([128, QW, V], BF16, tag="e2")
                    nc.vector.tensor_scalar(
                        e2[:], s[:], 2.0, 0.0, ALU.is_equal, ALU.add,
                        accum_out=out_sb[:, col + 1:col + 2])
                    gs = bcep.tile([128, QW, V], BF16, tag="gs")
                    nc.vector.tensor_tensor(gs[:], gt_sb[:, z0:z0 + QW],
                                            s[:], ALU.mult)
                    g2 = bcep.tile([128, QW, V], BF16, tag="g2")
                    nc.vector.tensor_scalar(
                        g2[:], gs[:], 0.0, 0.0, ALU.add, ALU.add,
                        accum_out=out_sb[:, col + 2:col + 3])

            FB = 2
            for k0 in range(0, ZW, FB):
                psV = psVp.tile([128, 2, FB, V], F32)
                for b in range(2):
                    nc.tensor.matmul(psV[:, b], lhsT=maskF(b),
                                     rhs=ATxF[:, k0:k0 + FB, :],
                                     start=True, stop=True)
                vsb = vsbp.tile([128, 2, FB, V], BF16, tag="v")
                nc.scalar.copy(vsb[:], psV[:])
                for kl in range(FB):
                    kk = k0 + kl
                    psC = psCFp.tile([128, 2, V], F32)
                    nc.tensor.matmul(psC[:], lhsT=ATyF[:, kk, :],
                                     rhs=vsb[:, :, kl, :],
                                     start=True, stop=True)
                    nc.scalar.sign(volF[:, :, kk, :], psC[:])
                if (k0 + FB) % QW == 0:
                    bce_quarter((k0 + FB) // QW - 1)
            nc.sync.dma_start(out_p.ap(), out_sb[:])

    nc.compile()
    _PROGRAM_CACHE[key] = nc
    return nc


def _host_prep(inputs):
    """Validate geometry and build per-core bf16 tables.

    Returns (in_maps, nl_ks).  Raises _GeometryFallback when the separability
    assumptions don't hold.
    """
    f32 = np.float32
    pf = np.asarray(inputs["pred_frontal"], dtype=f32)
    pl = np.asarray(inputs["pred_lateral"], dtype=f32)
    srcF = np.asarray(inputs["source_F"], dtype=np.float64)[0]
    tgtF = np.asarray(inputs["target_F"], dtype=np.float64)[0]
    srcL = np.asarray(inputs["source_L"], dtype=np.float64)[0]
    tgtL = np.asarray(inputs["target_L"], dtype=np.float64)[0]
    A_inv = np.asarray(inputs["A_inv"], dtype=np.float64)
    t_inv = np.asarray(inputs["t_inv"], dtype=np.float64)
    gt = np.asarray(inputs["vol_gt_3d"], dtype=f32)
    B = pf.shape[0]
    if B != 2 or gt.shape != (V, V, V) or pf.shape[2:] != (V, V):
        raise _GeometryFallback(f"unexpected shapes B={B}")
    if not np.array_equal(A_inv, np.diag(np.diag(A_inv))):
        raise _GeometryFallback("A_inv not diagonal")
    D = np.diag(A_inv)

    def view_tables(src, tgt, scan_ax, ax_i, ax_j):
        """Per-slice sample index + separable coordinate maps (f64)."""
        # target coordinate along scan axis must be globally constant;
        # along ax_i it may depend only on detector row i, ax_j only on j.
        c = tgt[0, 0, scan_ax]
        if not np.all(tgt[..., scan_ax] == c):
            raise _GeometryFallback("scan axis not constant")
        ti = tgt[:, 0, ax_i]
        if not np.all(tgt[..., ax_i] == ti[:, None]):
            raise _GeometryFallback("ax_i not separable")
        tj = tgt[0, :, ax_j]
        if not np.all(tgt[..., ax_j] == tj[None, :]):
            raise _GeometryFallback("ax_j not separable")

        beta = 2.5 * np.arange(S, dtype=np.float64) / (S - 1.0)
        zeta = (src[scan_ax] + (c - src[scan_ax]) * beta) * D[scan_ax] \
            + t_inv[scan_ax]
        ks = np.rint(zeta).astype(np.int64)
        margin = np.abs(np.abs(zeta - np.rint(zeta)) - 0.5).min()
        if margin < 5e-4:
            raise _GeometryFallback(f"scan margin {margin:.1e}")
        inb = (ks >= 0) & (ks < V)
        if len(np.unique(ks[inb])) != int(inb.sum()):
            raise _GeometryFallback("multiple samples per slice")
        s_for_k = np.full(V, -1, np.int64)
        s_for_k[ks[inb]] = np.arange(S)[inb]

        p = np.arange(V, dtype=np.float64)

        def cmap(tvals, axis):
            """[p, k] voxel coordinate map with POISON for invalid entries."""
            out = np.full((V, V), POISON, dtype=np.float64)
            for k in range(V):
                sk = s_for_k[k]
                if sk < 0:
                    continue
                a = beta[sk]
                w = (src[axis] + (tvals - src[axis]) * a) * D[axis] \
                    + t_inv[axis]
                m = np.abs(np.abs(w - np.rint(w)) - 0.5).min()
                if m < 5e-4:
                    raise _GeometryFallback(f"transverse margin {m:.1e}")
                r = np.rint(w)
                r[(r < 0) | (r >= V)] = POISON
                out[:, k] = r
            return out

        return s_for_k, cmap(ti, ax_i), cmap(tj, ax_j)

    # frontal: scan z(2), i -> vol axis 0 (x), j -> vol axis 1 (y)
    sfF, mapxF, mapyF = view_tables(srcF, tgtF, 2, 0, 1)
    # lateral: scan x(0), i -> vol axis 1 (y), j -> vol axis 2 (z)
    sfL, mapyL, mapzL = view_tables(srcL, tgtL, 0, 1, 2)

    nl_ks = tuple(int(k) for k in range(V) if sfL[k] >= 0)
    if not nl_ks:
        nl_ks = (0,)  # degenerate but keeps the program shape valid

    bf16 = ml_dtypes.bfloat16
    nl = len(nl_ks)
    c_vzl, c_mf, c_mlt, c_xv, tabs_w = _tab_offsets(nl)
    klist = np.array(nl_ks, dtype=np.int64)
    # x-axis permutation: the nl_ks slices first, the rest after
    xorder = np.concatenate(
        [klist, np.setdiff1d(np.arange(V, dtype=np.int64), klist)])
    maskF = (pf[:, 0] > 0.5)                       # [b, i, j]
    maskLT = (pl[:, 0] > 0.5).transpose(0, 2, 1)    # [b, j, i]
    gtzyx = np.ascontiguousarray(gt.transpose(1, 2, 0))  # [y][z][x]

    in_maps = []
    for cidx in range(N_CORES):
        z0 = ZW * cidx
        tabs = np.full((128, tabs_w), POISON, dtype=np.float64)
        tabs[:, _C_VXF:_C_VXF + ZW] = mapxF[:, z0:z0 + ZW]
        tabs[:, _C_VYF:_C_VYF + ZW] = mapyF[:, z0:z0 + ZW]
        tabs[:, _C_VYL:_C_VYL + nl] = mapyL[:, klist]
        vz = mapzL[:, klist].copy()
        ok = vz != POISON
        vz[ok] = vz[ok] - z0
        tabs[:, c_vzl:c_vzl + nl] = vz
        tabs[:, c_mf:c_mf + V] = maskF[0]
        tabs[:, c_mf + V:c_mf + 2 * V] = maskF[1]
        tabs[:, c_mlt:c_mlt + V] = maskLT[0]
        tabs[:, c_mlt + V:c_mlt + 2 * V] = maskLT[1]
        tabs[:, c_xv:c_xv + V] = xorder[None, :]
        gshard = gtzyx[:, z0:z0 + ZW, :][:, :, xorder].reshape(128, ZW * V)
        in_maps.append({"tabs": tabs.astype(bf16),
                        "gt": np.ascontiguousarray(gshard).astype(bf16)})
    return in_maps, nl_ks


def _combine(results) -> np.ndarray:
    """Host-side reduction of the 8 per-core [128, 8] partial-sum tensors."""
    acc = np.zeros(32, dtype=np.float64)
    for r in results:
        acc += np.asarray(r["out_vec"], dtype=np.float64).sum(axis=0)
    acc = acc.reshape(4, 8).sum(axis=0)   # fold the four z-quarters
    total = 0.0
    for b in range(2):
        ss = acc[4 * b] + acc[4 * b + 1]   # sum(s) = sum(min(s,1)) + sum(s==2)
        se2, sgs = acc[4 * b + 1], acc[4 * b + 2]
        total += Q0 * (V ** 3) + (Q1 + Q2) * ss + 2.0 * Q2 * se2 + sgs
    return np.float32(-total / (2.0 * V ** 3))


def _reference_fallback(inputs):
    """Faithful f32 numpy replica of the jax reference (safety net)."""
    f32 = np.float32
    pf = np.asarray(inputs["pred_frontal"], dtype=f32)
    pl = np.asarray(inputs["pred_lateral"], dtype=f32)
    srcF = np.asarray(inputs["source_F"], dtype=f32)[0]
    tgtF = np.asarray(inputs["target_F"], dtype=f32)[0]
    srcL = np.asarray(inputs["source_L"], dtype=f32)[0]
    tgtL = np.asarray(inputs["target_L"], dtype=f32)[0]
    A_inv = np.asarray(inputs["A_inv"], dtype=f32)
    t_inv = np.asarray(inputs["t_inv"], dtype=f32)
    gt = np.asarray(inputs["vol_gt_3d"], dtype=f32)

    def backproject(mask2d, src, tgt):
        active = (mask2d > 0.5).reshape(-1)
        det = tgt.reshape(-1, 3).astype(f32)
        rd = (det - src[None, :]).astype(f32)
        rl = np.sqrt((rd * rd).sum(1, dtype=f32)).astype(f32)[:, None]
        rdn = (rd / (rl + f32(1e-8))).astype(f32)
        tv = (np.arange(S, dtype=f32) * (f32(1.0) / f32(S - 1)))
        ts = (tv[None, :, None] * (rl[:, None, :] * f32(2.5))).astype(f32)
        world = (src[None, None, :] + rdn[:, None, :] * ts).astype(f32)
        vox_f = (world @ A_inv.T + t_inv).astype(f32)
        vox = np.rint(vox_f).astype(np.int64)
        ok = (active[:, None]
              & (vox[..., 0] >= 0) & (vox[..., 0] < V)
              & (vox[..., 1] >= 0) & (vox[..., 1] < V)
              & (vox[..., 2] >= 0) & (vox[..., 2] < V))
        vi = np.clip(vox, 0, V - 1)
        vol = np.zeros((V, V, V), dtype=f32)
        flat = (vi[..., 0] * V + vi[..., 1]) * V + vi[..., 2]
        vol.reshape(-1)[flat[ok]] = 1.0
        return vol

    total = 0.0
    B = pf.shape[0]
    for b in range(B):
        vF = backproject(pf[b, 0], srcF, tgtF)
        vL = backproject(pl[b, 0], srcL, tgtL)
        sv = (vF + vL).astype(np.float64)
        p = 1.0 / (1.0 + np.exp(-sv))
        total += -(gt * np.log(p) + (1.0 - gt) * np.log1p(-p)).mean()
    return np.float32(total / B)


def kernel(**inputs) -> np.ndarray:
    try:
        in_maps, nl_ks = _host_prep(inputs)
    except _GeometryFallback:
        return _reference_fallback(inputs)
    nc = _build_program(nl_ks)
    res = run_bass_kernel_spmd(nc, in_maps, list(range(N_CORES)))
    return _combine(res.results)


# revision 37
# speedup vs baseline: 1.8623x; 1.2957x over previous
"""Trainium2 Bass kernel: Backprojection3DConsistencyLoss (8-core SPMD).

Contract: kernel(**inputs) takes the FULL unsharded inputs of the reference
(pred_frontal/pred_lateral [2,1,128,128] f32, source/target geometry, the
ground-truth volume [128,128,128] f32, A_inv [3,3], t_inv [3]) and returns the
FULL scalar loss, computing the heavy work on 8 NeuronCores.

Algorithm (separable per-slice reconstruction; no collectives):
  For this module's geometry the detector plane is constant along the scan
  axis, so every ray shares the same scan-axis sample sequence
  z_s = src_z + dz*(2.5 s/511) (the ray-length normalization cancels exactly).
  Slice k therefore has a unique shared sample s_k (or none), and within the
  slice the hit voxel is a separable affine map of the detector indices:
  y = rint(sy + (j-sy)*a_k), x = rint(sx + (i-sx)*a_k) with a_k = 2.5 s_k/511.
  With one-hot matrices A_k[p, v] = [map_k(p) == v] the 0/1 slice image is
  sign(A_k^T . mask^T-ish . A_k) — two small matmuls on the TensorEngine.

  Sharding: core c owns z-slices [16c, 16c+16) of all four volumes.
  Frontal volumes are built slice-by-slice (z = slice axis); lateral volumes
  (x = slice axis) are built as the 16-column z-band of every x-slice, via a
  skinny pair of matmuls per non-empty x-slice.  All per-core variation is
  carried by host-computed bf16 lookup tables, so one SPMD program serves all
  8 cores and no ReduceScatter is needed.  Each core evaluates its BCE shard
  with the exact quadratic q0 + q1*s + q2*s^2 + gt*s (s in {0,1,2}) reduced
  on-device to a few per-partition sums; the host combines them.

If the input geometry violates the separability assumptions (it holds for
this module's detector geometry; margins are checked in f64), a faithful f32
numpy fallback computes the same result on host.
"""

import math
import sys

import numpy as np

for _p in ("/opt/trn_rl_repo",):
    if _p not in sys.path:
        sys.path.insert(0, _p)

import ml_dtypes  # noqa: E402

import concourse.bacc as bacc  # noqa: E402
import concourse.mybir as mybir  # noqa: E402
import concourse.tile as tile  # noqa: E402
from concourse.bass_utils import run_bass_kernel_spmd  # noqa: E402

N_CORES = 8
V = 128          # volume side
S = 512          # samples per ray
ZW = V // N_CORES  # z-slices per core (16)
POISON = 200.0   # one-hot compare value that never matches a voxel coordinate
F32 = mybir.dt.float32
BF16 = mybir.dt.bfloat16
I32 = mybir.dt.int32
ALU = mybir.AluOpType

# BCE quadratic: cell loss = q0 + q1*s + q2*s^2 + gt*s, exact for s in {0,1,2}
_B0 = math.log(0.5)
_B1 = -math.log1p(math.e)
_B2 = -2.0 - math.log1p(math.exp(-2.0))
Q0 = _B0
Q1 = (-3.0 * _B0 + 4.0 * _B1 - _B2) / 2.0
Q2 = (_B0 - 2.0 * _B1 + _B2) / 2.0

# packed bf16 table layout (columns of the [128, W] "tabs" input); lateral
# tables carry only the NL non-empty x-slices, in nl_ks order
_C_VXF = 0      # [128, 16]  frontal x map for the core's 16 z-slices
_C_VYF = 16     # [128, 16]  frontal y map
_C_VYL = 32     # [128, NL]  lateral y map
# _C_VZL = 32+NL  [128, NL]  lateral z map, shifted by -16c
# _C_MF  = 32+2NL [128, 2*128] frontal masks (i-partition, j-free)
# _C_MLT = +2*128 [128, 2*128] lateral masks transposed (j-part, i-free)


def _tab_offsets(nl):
    # one-hot build inputs (tables + xvals) first so their DMA can be issued
    # separately from (and ahead of) the mask columns
    c_vzl = _C_VYL + nl
    c_xv = c_vzl + nl
    c_mf = c_xv + V
    c_mlt = c_mf + 2 * V
    return c_vzl, c_mf, c_mlt, c_xv, c_mlt + 2 * V

_PROGRAM_CACHE: dict = {}


class _GeometryFallback(Exception):
    pass


def _build_program(nl_ks: tuple):
    key = nl_ks
    if key in _PROGRAM_CACHE:
        return _PROGRAM_CACHE[key]

    nl = len(nl_ks)
    c_vzl, c_mf, c_mlt, c_xv, tabs_w = _tab_offsets(nl)
    nc = bacc.Bacc("TRN2", target_bir_lowering=False, debug=False,
                   num_devices=N_CORES)
    tabs = nc.declare_dram_parameter("tabs", [128, tabs_w], BF16,
                                     isOutput=False)
    gt_p = nc.declare_dram_parameter("gt", [128, ZW * V], BF16, isOutput=False)
    out_p = nc.declare_dram_parameter("out_vec", [128, 32], F32, isOutput=True)

    with tile.TileContext(nc) as tc:
        with (
            tc.tile_pool(name="const", bufs=1) as constp,
            tc.tile_pool(name="vsb", bufs=3) as vsbp,
            tc.tile_pool(name="wsb", bufs=2) as wsbp,
            tc.tile_pool(name="bce", bufs=2) as bcep,
            tc.tile_pool(name="psV", bufs=2, space="PSUM") as psVp,
            tc.tile_pool(name="psCF", bufs=2, space="PSUM") as psCFp,
            tc.tile_pool(name="psW", bufs=2, space="PSUM") as psWp,
            tc.tile_pool(name="psC", bufs=2, space="PSUM") as psCp,
        ):
            tabs_sb = constp.tile([128, tabs_w], BF16)
            nc.sync.dma_start(tabs_sb[:, 0:c_mf], tabs.ap()[:, 0:c_mf])
            nc.sync.dma_start(tabs_sb[:, c_mf:], tabs.ap()[:, c_mf:])
            gt_sb = constp.tile([128, ZW, V], BF16)
            nc.sync.dma_start(gt_sb[:], gt_p.ap())

            # bf16 iota is exact for 0..127
            iota_b = constp.tile([128, V], BF16)
            nc.gpsimd.iota(iota_b[:], pattern=[[1, V]], base=0,
                           channel_multiplier=0,
                           allow_small_or_imprecise_dtypes=True)

            def one_hot(name, val_ap, nk, nv, eng=None):
                """AT[p, k, v] = (val[p, k] == v), bf16 {0,1}."""
                t = constp.tile([128, nk, nv], BF16, tag=name)
                (eng or nc.vector).tensor_tensor(
                    t[:],
                    iota_b[:, 0:nv].unsqueeze(1).broadcast_to([128, nk, nv]),
                    val_ap.unsqueeze(2).broadcast_to([128, nk, nv]),
                    ALU.is_equal)
                return t

            # lateral one-hots first: they gate the PE-dominant lateral pass.
            # ATyL is built in 16-slice chunks so the first lateral block's
            # mm2 isn't gated on the whole build.
            ATzL = one_hot("ATzL", tabs_sb[:, c_vzl:c_vzl + nl], nl, ZW)
            ATyL = constp.tile([128, nl, V], BF16, tag="ATyL")
            for p0 in range(0, nl, 16):
                pw = min(16, nl - p0)
                nc.vector.tensor_tensor(
                    ATyL[:, p0:p0 + pw, :],
                    iota_b[:].unsqueeze(1).broadcast_to([128, pw, V]),
                    tabs_sb[:, _C_VYL + p0:_C_VYL + p0 + pw]
                    .unsqueeze(2).broadcast_to([128, pw, V]),
                    ALU.is_equal)
            # ATxF compares against the permuted x-identity row (the volume
            # x-axis is stored with the nl_ks slices first, so lateral signs
            # can batch over contiguous columns)
            ATxF = constp.tile([128, ZW, V], BF16, tag="ATxF")
            nc.vector.tensor_tensor(
                ATxF[:],
                tabs_sb[:, c_xv:c_xv + V].unsqueeze(1)
                .broadcast_to([128, ZW, V]),
                tabs_sb[:, _C_VXF:_C_VXF + ZW].unsqueeze(2)
                .broadcast_to([128, ZW, V]),
                ALU.is_equal)
            ATyF = one_hot("ATyF", tabs_sb[:, _C_VYF:_C_VYF + ZW], ZW, V)

            def maskF(b):   # [i, j]
                return tabs_sb[:, c_mf + V * b:c_mf + V * (b + 1)]

            def maskLT(b):  # [j, i]
                return tabs_sb[:, c_mlt + V * b:c_mlt + V * (b + 1)]

            volF = constp.tile([128, 2, ZW, V], BF16)
            volL = constp.tile([128, 2, ZW, V], BF16)
            nc.gpsimd.memset(volL[:], 0.0)

            # ---- lateral: [y, z-band] columns of each non-empty x-slice,
            #      batched 16 slices per PSUM bank; mm1 is one wide matmul
            #      per (block, batch) since its weights (the mask) are fixed
            LB = 16
            for blk in range(0, nl, LB):
                ks = nl_ks[blk:blk + LB]
                nb = len(ks)
                psW = psWp.tile([128, 2, LB, ZW], F32)
                for b in range(2):
                    nc.tensor.matmul(psW[:, b, 0:nb, :], lhsT=maskLT(b),
                                     rhs=ATzL[:, blk:blk + nb, :],
                                     start=True, stop=True)
                wsb = wsbp.tile([128, 2, LB, ZW], BF16, tag="w")
                nc.scalar.copy(wsb[:, :, 0:nb, :], psW[:, :, 0:nb, :])
                psC2 = psCp.tile([128, LB, 2, ZW], F32)
                for slot, k in enumerate(ks):
                    nc.tensor.matmul(psC2[:, slot], lhsT=ATyL[:, blk + slot, :],
                                     rhs=wsb[:, :, slot, :],
                                     start=True, stop=True)
                # x-permuted layout: slices of this block sit in contiguous
                # volume columns [blk, blk+nb) -> one batched sign
                nc.scalar.sign(
                    volL[:, :, :, blk:blk + nb].transpose([0, 3, 1, 2]),
                    psC2[:, 0:nb])

            # ---- frontal: full [y, x] slice per owned z, mm1 two slices
            #      wide per batch.  The BCE is evaluated in four z-quarters
            #      interleaved with the frontal loop (the lateral volume is
            #      already complete), so the reduction runs in DVE idle time
            #      instead of as a serial tail.  out cols per (quarter c,
            #      batch b): 8c+4b+0 = sum(min(s,1)), +1 = sum(s==2),
            #      +2 = sum(gt*s); sum(s) = col0 + col1. ----
            out_sb = constp.tile([128, 32], F32)
            nc.gpsimd.memset(out_sb[:], 0.0)
            QW = ZW // 4   # z-slices per BCE quarter

            def bce_quarter(c):
                z0 = QW * c
                for b in range(2):
                    col = 8 * c + 4 * b
                    s = bcep.tile([128, QW, V], BF16, tag="s")
                    nc.vector.tensor_tensor(s[:], volF[:, b, z0:z0 + QW],
                                            volL[:, b, z0:z0 + QW], ALU.add)
                    s1 = bcep.tile([128, QW, V], BF16, tag="s1")
                    nc.vector.tensor_scalar(
                        s1[:], s[:], 1.0, 0.0, ALU.min, ALU.add,
                        accum_out=out_sb[:, col:col + 1])
                    e2 = bcep.tile([128, QW, V], BF16, tag="e2")
                    nc.vector.tensor_scalar(
                        e2[:], s[:], 2.0, 0.0, ALU.is_equal, ALU.add,
                        accum_out=out_sb[:, col + 1:col + 2])
                    gs = bcep.tile([128, QW, V], BF16, tag="gs")
                    nc.vector.tensor_tensor(gs[:], gt_sb[:, z0:z0 + QW],
                                            s[:], ALU.mult)
                    g2 = bcep.tile([128, QW, V], BF16, tag="g2")
                    nc.vector.tensor_scalar(
                        g2[:], gs[:], 0.0, 0.0, ALU.add, ALU.add,
                        accum_out=out_sb[:, col + 2:col + 3])

            FB = 2
            for k0 in range(0, ZW, FB):
                psV = psVp.tile([128, 2, FB, V], F32)
                for b in range(2):
                    nc.tensor.matmul(psV[:, b], lhsT=maskF(b),
                                     rhs=ATxF[:, k0:k0 + FB, :],
                                     start=True, stop=True)
                vsb = vsbp.tile([128, 2, FB, V], BF16, tag="v")
                nc.scalar.copy(vsb[:], psV[:])
                for kl in range(FB):
                    kk = k0 + kl
                    psC = psCFp.tile([128, 2, V], F32)
                    nc.tensor.matmul(psC[:], lhsT=ATyF[:, kk, :],
                                     rhs=vsb[:, :, kl, :],
                                     start=True, stop=True)
                    nc.scalar.sign(volF[:, :, kk, :], psC[:])
                if (k0 + FB) % QW == 0:
                    bce_quarter((k0 + FB) // QW - 1)
            nc.sync.dma_start(out_p.ap(), out_sb[:])

    nc.compile()
    _PROGRAM_CACHE[key] = nc
    return nc


def _host_prep(inputs):
    """Validate geometry and build per-core bf16 tables.

    Returns (in_maps, nl_ks).  Raises _GeometryFallback when the separability
    assumptions don't hold.
    """
    f32 = np.float32
    pf = np.asarray(inputs["pred_frontal"], dtype=f32)
    pl = np.asarray(inputs["pred_lateral"], dtype=f32)
    srcF = np.asarray(inputs["source_F"], dtype=np.float64)[0]
    tgtF = np.asarray(inputs["target_F"], dtype=np.float64)[0]
    srcL = np.asarray(inputs["source_L"], dtype=np.float64)[0]
    tgtL = np.asarray(inputs["target_L"], dtype=np.float64)[0]
    A_inv = np.asarray(inputs["A_inv"], dtype=np.float64)
    t_inv = np.asarray(inputs["t_inv"], dtype=np.float64)
    gt = np.asarray(inputs["vol_gt_3d"], dtype=f32)
    B = pf.shape[0]
    if B != 2 or gt.shape != (V, V, V) or pf.shape[2:] != (V, V):
        raise _GeometryFallback(f"unexpected shapes B={B}")
    if not np.array_equal(A_inv, np.diag(np.diag(A_inv))):
        raise _GeometryFallback("A_inv not diagonal")
    D = np.diag(A_inv)

    def view_tables(src, tgt, scan_ax, ax_i, ax_j):
        """Per-slice sample index + separable coordinate maps (f64)."""
        # target coordinate along scan axis must be globally constant;
        # along ax_i it may depend only on detector row i, ax_j only on j.
        c = tgt[0, 0, scan_ax]
        if not np.all(tgt[..., scan_ax] == c):
            raise _GeometryFallback("scan axis not constant")
        ti = tgt[:, 0, ax_i]
        if not np.all(tgt[..., ax_i] == ti[:, None]):
            raise _GeometryFallback("ax_i not separable")
        tj = tgt[0, :, ax_j]
        if not np.all(tgt[..., ax_j] == tj[None, :]):
            raise _GeometryFallback("ax_j not separable")

        beta = 2.5 * np.arange(S, dtype=np.float64) / (S - 1.0)
        zeta = (src[scan_ax] + (c - src[scan_ax]) * beta) * D[scan_ax] \
            + t_inv[scan_ax]
        ks = np.rint(zeta).astype(np.int64)
        margin = np.abs(np.abs(zeta - np.rint(zeta)) - 0.5).min()
        if margin < 5e-4:
            raise _GeometryFallback(f"scan margin {margin:.1e}")
        inb = (ks >= 0) & (ks < V)
        if len(np.unique(ks[inb])) != int(inb.sum()):
            raise _GeometryFallback("multiple samples per slice")
        s_for_k = np.full(V, -1, np.int64)
        s_for_k[ks[inb]] = np.arange(S)[inb]

        p = np.arange(V, dtype=np.float64)

        def cmap(tvals, axis):
            """[p, k] voxel coordinate map with POISON for invalid entries."""
            out = np.full((V, V), POISON, dtype=np.float64)
            for k in range(V):
                sk = s_for_k[k]
                if sk < 0:
                    continue
                a = beta[sk]
                w = (src[axis] + (tvals - src[axis]) * a) * D[axis] \
                    + t_inv[axis]
                m = np.abs(np.abs(w - np.rint(w)) - 0.5).min()
                if m < 5e-4:
                    raise _GeometryFallback(f"transverse margin {m:.1e}")
                r = np.rint(w)
                r[(r < 0) | (r >= V)] = POISON
                out[:, k] = r
            return out

        return s_for_k, cmap(ti, ax_i), cmap(tj, ax_j)

    # frontal: scan z(2), i -> vol axis 0 (x), j -> vol axis 1 (y)
    sfF, mapxF, mapyF = view_tables(srcF, tgtF, 2, 0, 1)
    # lateral: scan x(0), i -> vol axis 1 (y), j -> vol axis 2 (z)
    sfL, mapyL, mapzL = view_tables(srcL, tgtL, 0, 1, 2)

    nl_ks = tuple(int(k) for k in range(V) if sfL[k] >= 0)
    if not nl_ks:
        nl_ks = (0,)  # degenerate but keeps the program shape valid

    bf16 = ml_dtypes.bfloat16
    nl = len(nl_ks)
    c_vzl, c_mf, c_mlt, c_xv, tabs_w = _tab_offsets(nl)
    klist = np.array(nl_ks, dtype=np.int64)
    # x-axis permutation: the nl_ks slices first, the rest after
    xorder = np.concatenate(
        [klist, np.setdiff1d(np.arange(V, dtype=np.int64), klist)])
    maskF = (pf[:, 0] > 0.5)                       # [b, i, j]
    maskLT = (pl[:, 0] > 0.5).transpose(0, 2, 1)    # [b, j, i]
    gtzyx = np.ascontiguousarray(gt.transpose(1, 2, 0))  # [y][z][x]

    in_maps = []
    for cidx in range(N_CORES):
        z0 = ZW * cidx
        tabs = np.full((128, tabs_w), POISON, dtype=np.float64)
        tabs[:, _C_VXF:_C_VXF + ZW] = mapxF[:, z0:z0 + ZW]
        tabs[:, _C_VYF:_C_VYF + ZW] = mapyF[:, z0:z0 + ZW]
        tabs[:, _C_VYL:_C_VYL + nl] = mapyL[:, klist]
        vz = mapzL[:, klist].copy()
        ok = vz != POISON
        vz[ok] = vz[ok] - z0
        tabs[:, c_vzl:c_vzl + nl] = vz
        tabs[:, c_mf:c_mf + V] = maskF[0]
        tabs[:, c_mf + V:c_mf + 2 * V] = maskF[1]
        tabs[:, c_mlt:c_mlt + V] = maskLT[0]
        tabs[:, c_mlt + V:c_mlt + 2 * V] = maskLT[1]
        tabs[:, c_xv:c_xv + V] = xorder[None, :]
        gshard = gtzyx[:, z0:z0 + ZW, :][:, :, xorder].reshape(128, ZW * V)
        in_maps.append({"tabs": tabs.astype(bf16),
                        "gt": np.ascontiguousarray(gshard).astype(bf16)})
    return in_maps, nl_ks


def _combine(results) -> np.ndarray:
    """Host-side reduction of the 8 per-core [128, 8] partial-sum tensors."""
    acc = np.zeros(32, dtype=np.float64)
    for r in results:
        acc += np.asarray(r["out_vec"], dtype=np.float64).sum(axis=0)
    acc = acc.reshape(4, 8).sum(axis=0)   # fold the four z-quarters
    total = 0.0
    for b in range(2):
        ss = acc[4 * b] + acc[4 * b + 1]   # sum(s) = sum(min(s,1)) + sum(s==2)
        se2, sgs = acc[4 * b + 1], acc[4 * b + 2]
        total += Q0 * (V ** 3) + (Q1 + Q2) * ss + 2.0 * Q2 * se2 + sgs
    return np.float32(-total / (2.0 * V ** 3))


def _reference_fallback(inputs):
    """Faithful f32 numpy replica of the jax reference (safety net)."""
    f32 = np.float32
    pf = np.asarray(inputs["pred_frontal"], dtype=f32)
    pl = np.asarray(inputs["pred_lateral"], dtype=f32)
    srcF = np.asarray(inputs["source_F"], dtype=f32)[0]
    tgtF = np.asarray(inputs["target_F"], dtype=f32)[0]
    srcL = np.asarray(inputs["source_L"], dtype=f32)[0]
    tgtL = np.asarray(inputs["target_L"], dtype=f32)[0]
    A_inv = np.asarray(inputs["A_inv"], dtype=f32)
    t_inv = np.asarray(inputs["t_inv"], dtype=f32)
    gt = np.asarray(inputs["vol_gt_3d"], dtype=f32)

    def backproject(mask2d, src, tgt):
        active = (mask2d > 0.5).reshape(-1)
        det = tgt.reshape(-1, 3).astype(f32)
        rd = (det - src[None, :]).astype(f32)
        rl = np.sqrt((rd * rd).sum(1, dtype=f32)).astype(f32)[:, None]
        rdn = (rd / (rl + f32(1e-8))).astype(f32)
        tv = (np.arange(S, dtype=f32) * (f32(1.0) / f32(S - 1)))
        ts = (tv[None, :, None] * (rl[:, None, :] * f32(2.5))).astype(f32)
        world = (src[None, None, :] + rdn[:, None, :] * ts).astype(f32)
        vox_f = (world @ A_inv.T + t_inv).astype(f32)
        vox = np.rint(vox_f).astype(np.int64)
        ok = (active[:, None]
              & (vox[..., 0] >= 0) & (vox[..., 0] < V)
              & (vox[..., 1] >= 0) & (vox[..., 1] < V)
              & (vox[..., 2] >= 0) & (vox[..., 2] < V))
        vi = np.clip(vox, 0, V - 1)
        vol = np.zeros((V, V, V), dtype=f32)
        flat = (vi[..., 0] * V + vi[..., 1]) * V + vi[..., 2]
        vol.reshape(-1)[flat[ok]] = 1.0
        return vol

    total = 0.0
    B = pf.shape[0]
    for b in range(B):
        vF = backproject(pf[b, 0], srcF, tgtF)
        vL = backproject(pl[b, 0], srcL, tgtL)
        sv = (vF + vL).astype(np.float64)
        p = 1.0 / (1.0 + np.exp(-sv))
        total += -(gt * np.log(p) + (1.0 - gt) * np.log1p(-p)).mean()
    return np.float32(total / B)


def kernel(**inputs) -> np.ndarray:
    try:
        in_maps, nl_ks = _host_prep(inputs)
    except _GeometryFallback:
        return _reference_fallback(inputs)
    nc = _build_program(nl_ks)
    res = run_bass_kernel_spmd(nc, in_maps, list(range(N_CORES)))
    return _combine(res.results)
